# revision 1
# baseline (speedup 1.0000x reference)
"""Trainium2 Bass kernel for CantorAttention.

Strategy
--------
The Cantor routes are a pure function of the (quantized) Cantor value of each
position: sorting positions by that value makes every query's 64-key route set
live inside a narrow (<=385-wide) window of the sorted order.  Sparse
attention therefore becomes dense *banded* attention after a host-side
permutation:

  host:   pi = argsort(cantor_val), permute x rows, transpose; build per
          128-query-tile 128-aligned windows of width 384 plus an additive
          bf16 mask (-30000 at non-selected slots).
  device: qkvT projection (fp32r matmuls), banded scores + mask (PE),
          exp+rowsum (ACT, fused accum), normalize (GPSIMD), PE-transpose of
          the probabilities into per-128-chunk column-major buffers, PV
          matmuls accumulating transposed attention output, and the output
          projection producing a partial (4-head) outT block.
  host:   sum the 4 partial outT blocks per batch, transpose, un-permute,
          add the output bias.

Sharding: batch x head-block -> 8 cores (core c: b = c//4, heads 4*(c%4)..).
"""

import sys

sys.path.insert(0, "/opt/trn_rl_repo")

import numpy as np

B, S, DIM = 2, 2048, 1024
HEADS, DH = 16, 64
K_NEI = 64
N_CORES = 8
HPC = 4            # heads per core
QT = 128           # query tile (rows per tile)
NT = S // QT       # 16 query tiles
SUP = 4            # query tiles per supertile (PV batch of 512 queries)
NSUP = NT // SUP

_CACHE = {}


def _cantor_val(seq_len, depth=8):
    pos = np.arange(seq_len, dtype=np.float64)
    x = pos / max(1, seq_len - 1)
    x = np.clip(x, 1e-6, 1.0 - 1e-6)
    val = np.zeros_like(x)
    factor = 0.5
    for _ in range(depth):
        xs = x * 3.0
        digit = np.floor(xs)
        x = xs - digit
        val = val + (digit == 2.0).astype(np.float64) * factor
        factor *= 0.5
    return np.clip(val, 0.0, 1.0)


def _geometry(routes):
    """Window geometry from the runtime routes array."""
    val = _cantor_val(S)
    pi = np.argsort(val, kind="stable").astype(np.int64)
    rank = np.empty(S, np.int64)
    rank[pi] = np.arange(S)
    kr = rank[np.asarray(routes, np.int64)][pi]      # [S, K] key ranks, query-rank order
    lo = kr.min(1)
    hi = kr.max(1) + 1
    for win in (384, 512):
        a = np.zeros(NT, np.int64)
        ok = True
        for t in range(NT):
            l = int(lo[t * QT:(t + 1) * QT].min())
            h = int(hi[t * QT:(t + 1) * QT].max())
            a[t] = min(l // 128, (S - win) // 128)
            if h > a[t] * 128 + win:
                ok = False
                break
        if ok:
            return pi, rank, kr, a, win
    raise ValueError("routes structure incompatible with banded-window kernel")


def _build_module(a, win, loop_n=1, phases="ACD", cheat_dma=False):
    from contextlib import nullcontext

    from concourse import bacc, tile, mybir
    from concourse.masks import make_identity

    f32 = mybir.dt.float32
    f32r = mybir.dt.float32r
    bf16 = mybir.dt.bfloat16
    AF = mybir.ActivationFunctionType
    NCH = win // 128                      # chunks per window
    a = [int(v) for v in a]

    # chunk -> [first tile, last tile] using it
    chunk_tiles = {}
    for t in range(NT):
        for j in range(NCH):
            c = a[t] + j
            lo_t, hi_t = chunk_tiles.get(c, (t, t))
            chunk_tiles[c] = (min(lo_t, t), max(hi_t, t))

    nc = bacc.Bacc("TRN2", target_bir_lowering=False, debug=False)
    xT = nc.dram_tensor("xT", [DIM, S], f32r, kind="ExternalInput").ap()
    wq = nc.dram_tensor("wq", [DIM, 3 * HPC * DH], f32r, kind="ExternalInput").ap()
    bq = nc.dram_tensor("bq", [3 * HPC * DH, 1], f32, kind="ExternalInput").ap()
    wo = nc.dram_tensor("wo", [HPC * DH, DIM], f32r, kind="ExternalInput").ap()
    mask = nc.dram_tensor("mask", [QT, NT * win], bf16, kind="ExternalInput").ap()
    outp = nc.dram_tensor("outp", [DIM, S], f32, kind="ExternalOutput").ap()

    NQKV = 3 * HPC * DH                  # 768 rows of qkvT
    NMT = NQKV // 128                    # 6 row-tiles of qkvT

    with tile.TileContext(nc) as tc:
        with tc.tile_pool(name="persist", bufs=1) as pp:
            id32 = pp.tile([128, 128], f32)
            make_identity(nc, id32)
            id_r = pp.tile([128, 128], f32r)
            nc.vector.tensor_copy(id_r, id32)
            id_b = pp.tile([128, 128], bf16)
            nc.vector.tensor_copy(id_b, id32)
            mask_sb = pp.tile([QT, NT * win], bf16)
            nc.sync.dma_start(out=mask_sb, in_=mask)
            bq_sb = []
            for m in range(NMT):
                bt = pp.tile([128, 1], f32, tag=f"bq{m}", name=f"bq{m}")
                nc.sync.dma_start(out=bt, in_=bq[m * 128:(m + 1) * 128, :])
                bq_sb.append(bt)
            qkvT = [pp.tile([128, S], f32r, tag=f"qkvT{m}", name=f"qkvT{m}")
                    for m in range(NMT)]
            attn_outT = [pp.tile([128, S], f32r, tag=f"aout{p}", name=f"aout{p}")
                         for p in range(2)]
            wo_sb = []
            for p2 in range(2):
                wt = pp.tile([128, DIM], f32r, tag=f"wo{p2}", name=f"wo{p2}")
                nc.sync.dma_start(out=wt, in_=wo[p2 * 128:(p2 + 1) * 128, :])
                wo_sb.append(wt)

            loop_cm = tc.For_i(0, loop_n, 1) if loop_n > 1 else nullcontext()
            with loop_cm:
                # ------------- Phase A: qkvT = wq.T @ xT (+bias) -------------
                if "A" in phases:
                    with tc.tile_pool(name="phA", bufs=1) as pa, \
                         tc.tile_pool(name="phAx", bufs=2) as pax, \
                         tc.tile_pool(name="psA", bufs=3, space="PSUM") as psa:
                        wq_sb = []
                        for kk in range(8):
                            wt = pa.tile([128, NQKV], f32r, tag=f"wq{kk}",
                                         name=f"wq{kk}")
                            nc.sync.dma_start(out=wt, in_=wq[kk * 128:(kk + 1) * 128, :])
                            wq_sb.append(wt)
                        xt_prev = None
                        for n in range(4):
                            if cheat_dma and n > 0:
                                xt = xt_prev
                            else:
                                xt = []
                                for kk in range(8):
                                    t_ = pax.tile([128, 512], f32r, tag=f"x{kk}",
                                                  name=f"x{kk}_{n}")
                                    nc.sync.dma_start(
                                        out=t_,
                                        in_=xT[kk * 128:(kk + 1) * 128,
                                               n * 512:(n + 1) * 512])
                                    xt.append(t_)
                                xt_prev = xt
                            for m in (4, 5, 2, 3, 0, 1):
                                ps = psa.tile([128, 512], f32, tag="ps")
                                for kk in range(8):
                                    nc.tensor.matmul(
                                        ps, wq_sb[kk][:, m * 128:(m + 1) * 128], xt[kk],
                                        start=(kk == 0), stop=(kk == 7))
                                if (n + m) % 2 == 0:
                                    nc.scalar.activation(
                                        out=qkvT[m][:, n * 512:(n + 1) * 512],
                                        in_=ps, func=AF.Identity, bias=bq_sb[m])
                                else:
                                    nc.vector.tensor_scalar_add(
                                        qkvT[m][:, n * 512:(n + 1) * 512], ps,
                                        bq_sb[m])

                # ---------- Phases B+C: V transpose + banded attention ----------
                if "C" in phases:
                    with tc.tile_pool(name="phC", bufs=1) as pc, \
                         tc.tile_pool(name="pexp_pool", bufs=12) as pe_pool, \
                         tc.tile_pool(name="pt_pool", bufs=18) as pt_pool, \
                         tc.tile_pool(name="small", bufs=16) as sm_pool, \
                         tc.tile_pool(name="psB", bufs=3, space="PSUM") as psb, \
                         tc.tile_pool(name="psS", bufs=3, space="PSUM") as pss, \
                         tc.tile_pool(name="psO", bufs=2, space="PSUM") as pso:
                        V_sb = [pc.tile([128, 2 * 128], f32r, tag=f"V{cc}",
                                        name=f"V{cc}") for cc in range(NT)]
                        for cc in range(NT):
                            pv = psb.tile([128, 512], f32r, tag="ptr",
                                          name=f"pv{cc}")
                            for s_ in range(2):
                                nc.tensor.transpose(
                                    pv[:, s_ * 128:(s_ + 1) * 128],
                                    qkvT[4 + s_][:, cc * 128:(cc + 1) * 128], id_r)
                            if cc % 2 == 0:
                                nc.vector.tensor_copy(V_sb[cc], pv[:, 0:256])
                            else:
                                nc.scalar.copy(V_sb[cc], pv[:, 0:256])

                        aoutB = [pc.tile([64, S], f32r, tag=f"aoutB{i}",
                                         name=f"aoutB{i}") for i in range(2)]

                        def stage1(h, u, pn):
                            poff = (h % 2) * 64
                            qTh = qkvT[h // 2]
                            kTh = qkvT[2 + h // 2]
                            den_u = sm_pool.tile([128, SUP], f32, tag="den",
                                                 name=f"den{h}_{u}")
                            rec_u = sm_pool.tile([128, SUP], f32, tag="rec",
                                                 name=f"rec{h}_{u}")
                            pexps = {}
                            for t in range(u * SUP, (u + 1) * SUP):
                                w0 = a[t] * 128
                                ps_s = pss.tile([128, win], f32, tag="sc",
                                                name=f"sc{h}_{t}")
                                nc.tensor.matmul(
                                    ps_s,
                                    qTh[poff:poff + 64, t * 128:(t + 1) * 128],
                                    kTh[poff:poff + 64, w0:w0 + win],
                                    start=True, stop=False, skip_group_check=True)
                                nc.tensor.matmul(
                                    ps_s, id_b, mask_sb[:, t * win:(t + 1) * win],
                                    start=False, stop=True, skip_group_check=True)
                                pexp = pe_pool.tile([128, win], f32, tag="pexp",
                                                    name=f"pexp{h}_{t}")
                                i = t - u * SUP
                                nc.scalar.activation(out=pexp, in_=ps_s,
                                                     func=AF.Exp,
                                                     accum_out=den_u[:, i:i + 1])
                                pexps[t] = pexp
                            nc.vector.reciprocal(rec_u, den_u)
                            for t in range(u * SUP, (u + 1) * SUP):
                                i = t - u * SUP
                                pnorm = pe_pool.tile([128, win], f32r, tag="pnorm",
                                                     name=f"pnorm{h}_{t}")
                                nc.vector.tensor_scalar_mul(pnorm, pexps[t],
                                                            rec_u[:, i:i + 1])
                                pn[t] = pnorm

                        def stage2(h, u, pn, pt_tiles):
                            poff = (h % 2) * 64
                            # chunk-major transposes into a per-(chunk,unit) PSUM
                            # tile, then ONE copy per chunk into its SBUF buffer
                            tiles_u = range(u * SUP, (u + 1) * SUP)
                            cset = sorted({a[t] + j for t in tiles_u
                                           for j in range(NCH)})
                            for c in cset:
                                t0c, t1c = chunk_tiles[c]
                                if c not in pt_tiles:
                                    pt_tiles[c] = pt_pool.tile(
                                        [128, (t1c - t0c + 1) * 128], f32r,
                                        tag="pt", name=f"pt_h{h}_c{c}")
                                tlo = max(t0c, u * SUP)
                                thi = min(t1c, (u + 1) * SUP - 1)
                                wdt = (thi - tlo + 1) * 128
                                ptp = psb.tile([128, 512], f32r, tag="ptr",
                                               name=f"ptr{h}_{u}_{c}")
                                for t in range(tlo, thi + 1):
                                    nc.tensor.transpose(
                                        ptp[:, (t - tlo) * 128:(t - tlo + 1) * 128],
                                        pn[t][:, (c - a[t]) * 128:
                                              (c - a[t] + 1) * 128], id_r)
                                nc.vector.tensor_copy(
                                    pt_tiles[c][:, (tlo - t0c) * 128:
                                                (thi - t0c + 1) * 128],
                                    ptp[:, 0:wdt])
                            # PV pieces: widest chunk start=True, straddlers split
                            chunks_u = sorted({a[t] + j
                                               for t in range(u * SUP, (u + 1) * SUP)
                                               for j in range(NCH)})
                            ranges = []
                            for c in chunks_u:
                                t0c, t1c = chunk_tiles[c]
                                tlo = max(t0c, u * SUP)
                                thi = min(t1c, (u + 1) * SUP - 1)
                                ranges.append((c, tlo * 128 - u * 512,
                                               (thi + 1) * 128 - u * 512))
                            first = max(ranges, key=lambda r: r[2] - r[1])
                            pieces = [first]
                            wlo, whi = first[1], first[2]
                            for c, o0, o1 in sorted(
                                    (r for r in ranges if r is not first),
                                    key=lambda r: r[1]):
                                for p0, p1 in ((o0, min(o1, wlo)),
                                               (max(o0, wlo), min(o1, whi)),
                                               (max(o0, whi), o1)):
                                    if p1 > p0:
                                        pieces.append((c, p0, p1))
                                wlo, whi = min(wlo, o0), max(whi, o1)
                            po = pso.tile([128, 512], f32, tag="po",
                                          name=f"po{h}_{u}")
                            for i_p, (c, o0, o1) in enumerate(pieces):
                                t0c, _ = chunk_tiles[c]
                                r0 = o0 + u * 512 - t0c * 128
                                r1 = o1 + u * 512 - t0c * 128
                                nc.tensor.matmul(
                                    po[0:64, o0:o1],
                                    V_sb[c][:, h * 64:(h + 1) * 64],
                                    pt_tiles[c][:, r0:r1],
                                    start=(i_p == 0),
                                    stop=(i_p == len(pieces) - 1),
                                    skip_group_check=True)
                            if poff == 0:
                                dst = attn_outT[h // 2][0:64, u * 512:(u + 1) * 512]
                            else:
                                dst = aoutB[h // 2][:, u * 512:(u + 1) * 512]
                            nc.vector.tensor_copy(dst, po[0:64, :])
                            if poff != 0:
                                nc.sync.dma_start(
                                    out=attn_outT[h // 2][64:128,
                                                          u * 512:(u + 1) * 512],
                                    in_=aoutB[h // 2][:, u * 512:(u + 1) * 512])

                        units = [(h, u) for h in range(HPC) for u in range(NSUP)]
                        DELAY = 2
                        pn_store = {}
                        pt_store = {h: {} for h in range(HPC)}
                        pending = []
                        for h, u in units:
                            pn = {}
                            stage1(h, u, pn)
                            pn_store[(h, u)] = pn
                            pending.append((h, u))
                            if len(pending) > DELAY:
                                ph, pu = pending.pop(0)
                                stage2(ph, pu, pn_store.pop((ph, pu)), pt_store[ph])
                        for ph, pu in pending:
                            stage2(ph, pu, pn_store.pop((ph, pu)), pt_store[ph])

                # ------------- Phase D: outp = wo.T @ attn_outT -------------
                if "D" in phases:
                    with tc.tile_pool(name="phD", bufs=2) as pd, \
                         tc.tile_pool(name="psD", bufs=2, space="PSUM") as psd:
                        for mm in range(8):
                            st = pd.tile([128, S], f32, tag="st")
                            for n in range(4):
                                ps = psd.tile([128, 512], f32, tag="pod")
                                for p2 in range(2):
                                    nc.tensor.matmul(
                                        ps, wo_sb[p2][:, mm * 128:(mm + 1) * 128],
                                        attn_outT[p2][:, n * 512:(n + 1) * 512],
                                        start=(p2 == 0), stop=(p2 == 1))
                                if (mm + n) % 2 == 0:
                                    nc.scalar.copy(st[:, n * 512:(n + 1) * 512], ps)
                                else:
                                    nc.vector.tensor_copy(st[:, n * 512:(n + 1) * 512],
                                                          ps)
                            nc.sync.dma_start(out=outp[mm * 128:(mm + 1) * 128, :],
                                              in_=st)

    nc.compile()
    return nc


def _get_module(a, win):
    key = (tuple(int(v) for v in a), int(win))
    if key not in _CACHE:
        _CACHE[key] = _build_module(a, win)
    return _CACHE[key]


def kernel(x, routes, qkv_w, qkv_b, out_w, out_b):
    import ml_dtypes
    from concourse.bass_utils import run_bass_kernel_spmd

    x = np.ascontiguousarray(np.asarray(x, np.float32))
    routes = np.asarray(routes)
    qkv_w = np.asarray(qkv_w, np.float32)
    qkv_b = np.asarray(qkv_b, np.float32)
    out_w = np.asarray(out_w, np.float32)
    out_b = np.asarray(out_b, np.float32)

    pi, rank, kr, a, win = _geometry(routes)
    SCALE = 1.0 / float(np.sqrt(DH))

    # masks [QT, NT*win] additive bf16, shared by all cores
    mask_np = np.full((NT, QT, win), -30000.0, np.float32)
    rows = np.repeat(np.arange(QT), K_NEI)
    for t in range(NT):
        krt = (kr[t * QT:(t + 1) * QT] - a[t] * 128).ravel()
        mask_np[t, rows, krt] = 0.0
    mask_np = np.ascontiguousarray(
        mask_np.transpose(1, 0, 2).reshape(QT, NT * win)).astype(ml_dtypes.bfloat16)

    xT_b = [np.ascontiguousarray(x[b][pi].T) for b in range(B)]

    in_maps = []
    for c in range(N_CORES):
        b = c // (N_CORES // B)
        hb = c % (N_CORES // B)
        heads = range(hb * HPC, (hb + 1) * HPC)
        w_rows = []
        b_rows = []
        for sect, scale in ((0, SCALE), (1, 1.0), (2, 1.0)):
            for h in heads:
                r0 = sect * DIM + h * DH
                w_rows.append(qkv_w[r0:r0 + DH] * scale)
                b_rows.append(qkv_b[r0:r0 + DH] * scale)
        wq_c = np.ascontiguousarray(np.concatenate(w_rows, 0).T)          # [DIM, 768]
        bq_c = np.concatenate(b_rows, 0).reshape(-1, 1).astype(np.float32)
        wo_c = np.ascontiguousarray(out_w[:, hb * HPC * DH:(hb + 1) * HPC * DH].T)
        in_maps.append({
            "xT": xT_b[b],
            "wq": wq_c,
            "bq": bq_c,
            "wo": wo_c,
            "mask": mask_np,
        })

    nc = _get_module(a, win)
    res = run_bass_kernel_spmd(nc, in_maps, core_ids=list(range(N_CORES)))

    out = np.empty((B, S, DIM), np.float32)
    for b in range(B):
        cores = [c for c in range(N_CORES) if c // (N_CORES // B) == b]
        outT = res.results[cores[0]]["outp"].astype(np.float32)
        for c in cores[1:]:
            outT = outT + res.results[c]["outp"]
        rows_sorted = outT.T                      # [S, DIM] in rank order
        tmp = np.empty_like(rows_sorted)
        tmp[pi] = rows_sorted
        out[b] = tmp + out_b[None, :]
    return out



# revision 3
# speedup vs baseline: 1.2755x; 1.2755x over previous
"""Trainium2 Bass kernel for CantorAttention (transposed-softmax scheme).

Strategy
--------
Sorting positions by Cantor value makes every query's 64-key route set live
inside a 384-wide, 128-aligned window of the sorted order (dense banded
attention after a host-side permutation).  All matmuls run in bf16 (fp32
psum accumulation); rel err ~4e-3 vs the 2e-2 gate.

Per core (batch b, 4-head block hb), everything feature-major ("T" = [feat, seq]):

  A:  qkT = wqk.T @ xT (+bias, ACT copy to bf16); V produced directly in
      [seq, head, dim] layout (stationary = xT chunk, moving = wv), with a
      per-head ones column appended -> V65 [128, 4, 65].
  C:  scoresT per (head, 128-query tile): mask matmul FIRST (start=True,
      whole [128,384] psum = one zero-region), then 3 K^T.Q chunk matmuls
      accumulate.  ACT exp writes bf16 straight into a chunk-major E store
      (col = 384*chunk + 128*tile, collision-free for this geometry) via a
      strided AP -- no PE transposes, no DVE prob copies.
  PV: lhsT = V65[c][:,h,:] (65 rows: 64 v-dims + ones), rhs = E slices ->
      po[65, 512] = [unnormalized attn outT; denominators].  Normalize after:
      ACT copy po->SBUF, DVE reciprocal of the den row, Pool (gpsimd)
      partition_broadcast, DVE multiply -> attn_outT bf16.
  D:  outp = wo.T @ attn_outT per 512-col block, interleaved with C so the
      tail is short.

Sharding: batch x head-block -> 8 cores (core c: b = c//4, heads 4*(c%4)..).
Host sums the 4 partial outT blocks per batch, transposes, un-permutes, +bias.
"""

import sys

sys.path.insert(0, "/opt/trn_rl_repo")

import numpy as np

B, S, DIM = 2, 2048, 1024
HEADS, DH = 16, 64
K_NEI = 64
N_CORES = 8
HPC = 4            # heads per core
QT = 128           # query tile (rows per tile)
NT = S // QT       # 16 query tiles
SUP = 4            # query tiles per supertile (512 queries)
NSUP = NT // SUP

_CACHE = {}


def _cantor_val(seq_len, depth=8):
    pos = np.arange(seq_len, dtype=np.float64)
    x = pos / max(1, seq_len - 1)
    x = np.clip(x, 1e-6, 1.0 - 1e-6)
    val = np.zeros_like(x)
    factor = 0.5
    for _ in range(depth):
        xs = x * 3.0
        digit = np.floor(xs)
        x = xs - digit
        val = val + (digit == 2.0).astype(np.float64) * factor
        factor *= 0.5
    return np.clip(val, 0.0, 1.0)


def _geometry(routes):
    """Window geometry from the runtime routes array."""
    val = _cantor_val(S)
    pi = np.argsort(val, kind="stable").astype(np.int64)
    rank = np.empty(S, np.int64)
    rank[pi] = np.arange(S)
    kr = rank[np.asarray(routes, np.int64)][pi]      # [S, K] key ranks, query-rank order
    lo = kr.min(1)
    hi = kr.max(1) + 1
    for win in (384, 512):
        a = np.zeros(NT, np.int64)
        ok = True
        for t in range(NT):
            l = int(lo[t * QT:(t + 1) * QT].min())
            h = int(hi[t * QT:(t + 1) * QT].max())
            a[t] = min(l // 128, (S - win) // 128)
            if h > a[t] * 128 + win:
                ok = False
                break
        if ok:
            return pi, rank, kr, a, win
    raise ValueError("routes structure incompatible with banded-window kernel")


def _chunk_meta(a, win):
    """chunk -> (first tile, last tile); E-store block layout col=NCH*c + t."""
    NCH = win // 128
    a = [int(v) for v in a]
    chunk_tiles = {}
    for t in range(NT):
        for j in range(NCH):
            c = a[t] + j
            lo_t, hi_t = chunk_tiles.get(c, (t, t))
            chunk_tiles[c] = (min(lo_t, t), max(hi_t, t))
    blocks = {(c, t) for t in range(NT) for j in range(NCH) for c in (a[t] + j,)}
    idx = {NCH * c + t for (c, t) in blocks}
    if len(idx) != len(blocks):
        raise ValueError("E-store linear layout collision for this geometry")
    e_blocks = max(idx) + 1
    return chunk_tiles, e_blocks


def _build_module(a, win, loop_n=1, phases="ACD", cheat_dma=False):
    from contextlib import nullcontext

    from concourse import bacc, tile, mybir
    from concourse.bass import AP
    from concourse.masks import make_identity

    f32 = mybir.dt.float32
    bf16 = mybir.dt.bfloat16
    AF = mybir.ActivationFunctionType
    ALU = mybir.AluOpType
    NCH = win // 128                      # chunks per window
    KAP = win                             # E-store chunk stride (cols)
    a = [int(v) for v in a]
    chunk_tiles, e_blocks = _chunk_meta(a, win)
    E_COLS = e_blocks * 128

    nc = bacc.Bacc("TRN2", target_bir_lowering=False, debug=False)
    xT = nc.dram_tensor("xT", [DIM, S], bf16, kind="ExternalInput").ap()
    wqk = nc.dram_tensor("wqk", [DIM, 512], bf16, kind="ExternalInput").ap()
    bqk = nc.dram_tensor("bqk", [512, 1], f32, kind="ExternalInput").ap()
    wv = nc.dram_tensor("wv", [DIM, 256], bf16, kind="ExternalInput").ap()
    bv = nc.dram_tensor("bv", [1, 256], bf16, kind="ExternalInput").ap()
    wo = nc.dram_tensor("wo", [256, DIM], bf16, kind="ExternalInput").ap()
    maskT = nc.dram_tensor("maskT", [128, NT * win], bf16, kind="ExternalInput").ap()
    outp = nc.dram_tensor("outp", [DIM, S], f32, kind="ExternalOutput").ap()

    with tile.TileContext(nc) as tc:
        with tc.tile_pool(name="persist", bufs=1) as pp:
            id32 = pp.tile([128, 128], f32)
            make_identity(nc, id32)
            id_b = pp.tile([128, 128], bf16)
            nc.vector.tensor_copy(id_b, id32)
            ones1 = pp.tile([1, 128], bf16)
            nc.vector.memset(ones1, 1.0)

            # weights first (phase A gating), then mask (phase C), wo (phase D)
            wqk_sb = []
            for kk in range(8):
                wt = pp.tile([128, 512], bf16, tag=f"wqk{kk}", name=f"wqk{kk}")
                nc.sync.dma_start(out=wt, in_=wqk[kk * 128:(kk + 1) * 128, :])
                wqk_sb.append(wt)
            bq_sb = []
            for m in range(4):
                bt = pp.tile([128, 1], f32, tag=f"bqk{m}", name=f"bqk{m}")
                nc.sync.dma_start(out=bt, in_=bqk[m * 128:(m + 1) * 128, :])
                bq_sb.append(bt)
            wv_sb = []
            for kk in range(8):
                wt = pp.tile([128, 256], bf16, tag=f"wv{kk}", name=f"wv{kk}")
                nc.sync.dma_start(out=wt, in_=wv[kk * 128:(kk + 1) * 128, :])
                wv_sb.append(wt)
            bv_sb = pp.tile([1, 256], bf16)
            nc.sync.dma_start(out=bv_sb, in_=bv)
            maskT_sb = pp.tile([128, NT * win], bf16)
            nc.sync.dma_start(out=maskT_sb, in_=maskT)
            wo_sb = []
            for p2 in range(2):
                wt = pp.tile([128, DIM], bf16, tag=f"wo{p2}", name=f"wo{p2}")
                nc.sync.dma_start(out=wt, in_=wo[p2 * 128:(p2 + 1) * 128, :])
                wo_sb.append(wt)

            qk_sb = [pp.tile([128, S], bf16, tag=f"qk{m}", name=f"qk{m}")
                     for m in range(4)]
            V65 = [pp.tile([128, HPC, 65], bf16, tag=f"V{cc}", name=f"V{cc}")
                   for cc in range(NT)]
            E_st = [pp.tile([128, E_COLS], bf16, tag=f"E{h}", name=f"E{h}")
                    for h in range(HPC)]
            attn_outT = [pp.tile([128, S], bf16, tag=f"aout{p}", name=f"aout{p}")
                         for p in range(2)]

            loop_cm = tc.For_i(0, loop_n, 1) if loop_n > 1 else nullcontext()
            with loop_cm:
                with tc.tile_pool(name="xt_pool", bufs=2) as pax, \
                     tc.tile_pool(name="st_pool", bufs=3) as pst, \
                     tc.tile_pool(name="ao_pool", bufs=3) as pao, \
                     tc.tile_pool(name="rec_pool", bufs=2) as prc, \
                     tc.tile_pool(name="prb_pool", bufs=2) as prb_p, \
                     tc.tile_pool(name="psAV", bufs=3, space="PSUM") as psav, \
                     tc.tile_pool(name="psS", bufs=3, space="PSUM") as pss, \
                     tc.tile_pool(name="psPO", bufs=2, space="PSUM") as pspo:

                    xt_hold = [None]

                    def emit_A(n):
                        if "A" not in phases:
                            return
                        if cheat_dma and xt_hold[0] is not None:
                            xt = xt_hold[0]
                        else:
                            xt = []
                            for kk in range(8):
                                t_ = pax.tile([128, 512], bf16, tag=f"x{kk}",
                                              name=f"x{kk}_{n}")
                                nc.sync.dma_start(
                                    out=t_,
                                    in_=xT[kk * 128:(kk + 1) * 128,
                                           n * 512:(n + 1) * 512])
                                xt.append(t_)
                            xt_hold[0] = xt
                        for m in range(4):
                            ps = psav.tile([128, 512], f32, tag="av",
                                           name=f"psqk{m}_{n}")
                            for kk in range(8):
                                nc.tensor.matmul(
                                    ps, wqk_sb[kk][:, m * 128:(m + 1) * 128],
                                    xt[kk], start=(kk == 0), stop=(kk == 7))
                            nc.scalar.activation(
                                out=qk_sb[m][:, n * 512:(n + 1) * 512],
                                in_=ps, func=AF.Identity, bias=bq_sb[m])
                        for ss in range(4):
                            cc = n * 4 + ss
                            ps = psav.tile([128, 512], f32, tag="av",
                                           name=f"psv{cc}")
                            psv = ps[:, 0:256]
                            for kk in range(8):
                                nc.tensor.matmul(
                                    psv, xt[kk][:, ss * 128:(ss + 1) * 128],
                                    wv_sb[kk], start=(kk == 0), stop=False,
                                    skip_group_check=True)
                            nc.tensor.matmul(
                                psv, ones1, bv_sb, start=False, stop=True,
                                skip_group_check=True)
                            nc.vector.tensor_copy(V65[cc][:, :, 0:64],
                                                  psv.rearrange("p (h d) -> p h d", h=4))
                            nc.gpsimd.memset(V65[cc][:, :, 64:65], 1.0)

                    def emit_s1(h, u):
                        qTh = qk_sb[h // 2]
                        kTh = qk_sb[2 + h // 2]
                        poff = (h % 2) * 64
                        for t in range(u * SUP, (u + 1) * SUP):
                            ps = pss.tile([128, 512], f32, tag="sc",
                                          name=f"sc{h}_{t}")
                            ps_s = ps[:, 0:win]
                            nc.tensor.matmul(
                                ps_s, id_b, maskT_sb[:, t * win:(t + 1) * win],
                                start=True, stop=False, skip_group_check=True)
                            for j in range(NCH):
                                nc.tensor.matmul(
                                    ps_s[:, j * 128:(j + 1) * 128],
                                    kTh[poff:poff + 64,
                                        (a[t] + j) * 128:(a[t] + j + 1) * 128],
                                    qTh[poff:poff + 64, t * 128:(t + 1) * 128],
                                    start=False, stop=(j == NCH - 1),
                                    skip_group_check=True)
                            # exp -> E store, one strided op: block col = NCH*c + t
                            col0 = (NCH * a[t] + t) * 128
                            base = E_st[h][:, col0:col0 + 128]
                            out_ap = AP(base.tensor, base.offset,
                                        [list(base.ap[0]), [KAP, NCH], [1, 128]])
                            nc.scalar.activation(out=out_ap, in_=ps_s, func=AF.Exp)

                    def emit_s2(h, u):
                        # PV pieces: widest chunk start=True, straddlers split
                        tiles_u = range(u * SUP, (u + 1) * SUP)
                        chunks_u = sorted({a[t] + j for t in tiles_u
                                           for j in range(NCH)})
                        ranges = []
                        for c in chunks_u:
                            t0c, t1c = chunk_tiles[c]
                            tlo = max(t0c, u * SUP)
                            thi = min(t1c, (u + 1) * SUP - 1)
                            ranges.append((c, tlo * 128 - u * 512,
                                           (thi + 1) * 128 - u * 512))
                        first = max(ranges, key=lambda r: r[2] - r[1])
                        pieces = [first]
                        wlo, whi = first[1], first[2]
                        for c, o0, o1 in sorted(
                                (r for r in ranges if r is not first),
                                key=lambda r: r[1]):
                            for p0, p1 in ((o0, min(o1, wlo)),
                                           (max(o0, wlo), min(o1, whi)),
                                           (max(o0, whi), o1)):
                                if p1 > p0:
                                    pieces.append((c, p0, p1))
                            wlo, whi = min(wlo, o0), max(whi, o1)
                        po = pspo.tile([65, 512], f32, tag="po",
                                       name=f"po{h}_{u}")
                        for i_p, (c, o0, o1) in enumerate(pieces):
                            e0 = KAP * c + o0 + u * 512
                            nc.tensor.matmul(
                                po[:, o0:o1],
                                V65[c][:, h, :],
                                E_st[h][:, e0:e0 + (o1 - o0)],
                                start=(i_p == 0),
                                stop=(i_p == len(pieces) - 1),
                                skip_group_check=True)
                        ao = pao.tile([65, 512], f32, tag="ao",
                                      name=f"ao{h}_{u}")
                        nc.scalar.copy(ao, po)
                        rec = prc.tile([1, 512], f32, tag="rec",
                                       name=f"rec{h}_{u}")
                        nc.vector.reciprocal(rec, ao[64:65, :])
                        prb = prb_p.tile([64, 512], f32, tag="prb",
                                         name=f"prb{h}_{u}")
                        nc.gpsimd.partition_broadcast(prb, rec)
                        poff = (h % 2) * 64
                        nc.vector.tensor_tensor(
                            attn_outT[h // 2][poff:poff + 64,
                                              u * 512:(u + 1) * 512],
                            ao[0:64, :], prb, ALU.mult)

                    def emit_D(nb):
                        if "D" not in phases:
                            return
                        for mm in range(8):
                            ps = psav.tile([128, 512], f32, tag="av",
                                           name=f"psd{mm}_{nb}")
                            for p2 in range(2):
                                nc.tensor.matmul(
                                    ps, wo_sb[p2][:, mm * 128:(mm + 1) * 128],
                                    attn_outT[p2][:, nb * 512:(nb + 1) * 512],
                                    start=(p2 == 0), stop=(p2 == 1))
                            st = pst.tile([128, 512], f32, tag="st",
                                          name=f"st{mm}_{nb}")
                            if (mm + nb) % 2 == 0:
                                nc.scalar.copy(st, ps)
                            else:
                                nc.vector.tensor_copy(st, ps)
                            nc.sync.dma_start(
                                out=outp[mm * 128:(mm + 1) * 128,
                                         nb * 512:(nb + 1) * 512],
                                in_=st)

                    emit_A(0)
                    emit_A(1)
                    if "C" in phases:
                        for h in range(HPC):
                            emit_s1(h, 0)
                        emit_A(2)
                        for h in range(HPC):
                            emit_s2(h, 0)
                            emit_s1(h, 1)
                        emit_A(3)
                        for h in range(HPC):
                            emit_s2(h, 1)
                            emit_s1(h, 2)
                        emit_D(0)
                        for h in range(HPC):
                            emit_s2(h, 2)
                            emit_s1(h, 3)
                        emit_D(1)
                        for h in range(HPC):
                            emit_s2(h, 3)
                        emit_D(2)
                        emit_D(3)
                    else:
                        emit_A(2)
                        emit_A(3)

    nc.compile()
    return nc


def _get_module(a, win):
    key = (tuple(int(v) for v in a), int(win))
    if key not in _CACHE:
        _CACHE[key] = _build_module(a, win)
    return _CACHE[key]


def _prepare_in_maps(x, routes, qkv_w, qkv_b, out_w, out_b):
    """Shared host-side prep: returns (in_maps, pi, a, win)."""
    import ml_dtypes

    bf = ml_dtypes.bfloat16
    x = np.ascontiguousarray(np.asarray(x, np.float32))
    qkv_w = np.asarray(qkv_w, np.float32)
    qkv_b = np.asarray(qkv_b, np.float32)
    out_w = np.asarray(out_w, np.float32)

    pi, rank, kr, a, win = _geometry(np.asarray(routes))
    NCH = win // 128
    SCALE = 1.0 / float(np.sqrt(DH))

    # maskT [128, NT*win]: maskT[p, t*win + j*128 + q] = mask for
    # key (a[t]+j)*128+p, query t*128+q (0 selected / -30000 not)
    mask3 = np.full((NT, QT, win), -30000.0, np.float32)
    rows = np.repeat(np.arange(QT), K_NEI)
    for t in range(NT):
        krt = (kr[t * QT:(t + 1) * QT] - a[t] * 128).ravel()
        mask3[t, rows, krt] = 0.0
    maskT_np = np.ascontiguousarray(
        mask3.reshape(NT, QT, NCH, 128).transpose(3, 0, 2, 1).reshape(128, NT * win)
    ).astype(bf)

    xT_b = [np.ascontiguousarray(x[b][pi].T).astype(bf) for b in range(B)]

    in_maps = []
    for c in range(N_CORES):
        b = c // (N_CORES // B)
        hb = c % (N_CORES // B)
        heads = range(hb * HPC, (hb + 1) * HPC)
        w_rows, b_rows = [], []
        for sect, scale in ((0, SCALE), (1, 1.0)):
            for h in heads:
                r0 = sect * DIM + h * DH
                w_rows.append(qkv_w[r0:r0 + DH] * scale)
                b_rows.append(qkv_b[r0:r0 + DH] * scale)
        wqk_c = np.ascontiguousarray(np.concatenate(w_rows, 0).T).astype(bf)
        bqk_c = np.concatenate(b_rows, 0).reshape(-1, 1).astype(np.float32)
        vr0 = 2 * DIM + hb * HPC * DH
        wv_c = np.ascontiguousarray(qkv_w[vr0:vr0 + 256].T).astype(bf)
        bv_c = np.ascontiguousarray(qkv_b[vr0:vr0 + 256].reshape(1, 256)).astype(bf)
        wo_c = np.ascontiguousarray(
            out_w[:, hb * HPC * DH:(hb + 1) * HPC * DH].T).astype(bf)
        in_maps.append({
            "xT": xT_b[b],
            "wqk": wqk_c,
            "bqk": bqk_c,
            "wv": wv_c,
            "bv": bv_c,
            "wo": wo_c,
            "maskT": maskT_np,
        })
    return in_maps, pi, a, win


def kernel(x, routes, qkv_w, qkv_b, out_w, out_b):
    from concourse.bass_utils import run_bass_kernel_spmd

    out_b = np.asarray(out_b, np.float32)
    in_maps, pi, a, win = _prepare_in_maps(x, routes, qkv_w, qkv_b, out_w, out_b)

    nc = _get_module(a, win)
    res = run_bass_kernel_spmd(nc, in_maps, core_ids=list(range(N_CORES)))

    out = np.empty((B, S, DIM), np.float32)
    for b in range(B):
        cores = [c for c in range(N_CORES) if c // (N_CORES // B) == b]
        outT = res.results[cores[0]]["outp"].astype(np.float32)
        for c in cores[1:]:
            outT = outT + res.results[c]["outp"]
        rows_sorted = outT.T                      # [S, DIM] in rank order
        tmp = np.empty_like(rows_sorted)
        tmp[pi] = rows_sorted
        out[b] = tmp + out_b[None, :]
    return out


# revision 5
# speedup vs baseline: 1.4152x; 1.1095x over previous
"""Trainium2 Bass kernel for CantorAttention (transposed-softmax scheme).

Strategy
--------
Sorting positions by Cantor value makes every query's 64-key route set live
inside a 384-wide, 128-aligned window of the sorted order (dense banded
attention after a host-side permutation).  All matmuls run in bf16 (fp32
psum accumulation); rel err ~4e-3 vs the 2e-2 gate.

Per core (batch b, 4-head block hb), everything feature-major ("T" = [feat, seq]):

  A:  qkT = wqk.T @ xT (+bias, ACT copy to bf16); V produced directly in
      [seq, head, dim] layout (stationary = xT chunk, moving = wv), with a
      per-head ones column appended -> V65 [128, 4, 65].
  C:  scoresT per (head, 128-query tile): mask matmul FIRST (start=True,
      whole [128,384] psum = one zero-region), then 3 K^T.Q chunk matmuls
      accumulate.  ACT exp writes bf16 straight into a chunk-major E store
      (col = 384*chunk + 128*tile, collision-free for this geometry) via a
      strided AP -- no PE transposes, no DVE prob copies.
  PV: lhsT = V65[c][:,h,:] (65 rows: 64 v-dims + ones), rhs = E slices ->
      po[65, 512] = [unnormalized attn outT; denominators].  Normalize after:
      ACT copy po->SBUF, DVE reciprocal of the den row, Pool (gpsimd)
      partition_broadcast, DVE multiply -> attn_outT bf16.
  D:  outp = wo.T @ attn_outT per 512-col block, interleaved with C so the
      tail is short.

Sharding: batch x head-block -> 8 cores (core c: b = c//4, heads 4*(c%4)..).
Host sums the 4 partial outT blocks per batch, transposes, un-permutes, +bias.
"""

import sys

sys.path.insert(0, "/opt/trn_rl_repo")

import numpy as np

B, S, DIM = 2, 2048, 1024
HEADS, DH = 16, 64
K_NEI = 64
N_CORES = 8
HPC = 4            # heads per core
QT = 128           # query tile (rows per tile)
NT = S // QT       # 16 query tiles
SUP = 4            # query tiles per supertile (512 queries)
NSUP = NT // SUP

_CACHE = {}


def _cantor_val(seq_len, depth=8):
    pos = np.arange(seq_len, dtype=np.float64)
    x = pos / max(1, seq_len - 1)
    x = np.clip(x, 1e-6, 1.0 - 1e-6)
    val = np.zeros_like(x)
    factor = 0.5
    for _ in range(depth):
        xs = x * 3.0
        digit = np.floor(xs)
        x = xs - digit
        val = val + (digit == 2.0).astype(np.float64) * factor
        factor *= 0.5
    return np.clip(val, 0.0, 1.0)


def _geometry(routes):
    """Window geometry from the runtime routes array."""
    val = _cantor_val(S)
    pi = np.argsort(val, kind="stable").astype(np.int64)
    rank = np.empty(S, np.int64)
    rank[pi] = np.arange(S)
    kr = rank[np.asarray(routes, np.int64)][pi]      # [S, K] key ranks, query-rank order
    lo = kr.min(1)
    hi = kr.max(1) + 1
    for win in (384, 512):
        a = np.zeros(NT, np.int64)
        ok = True
        for t in range(NT):
            l = int(lo[t * QT:(t + 1) * QT].min())
            h = int(hi[t * QT:(t + 1) * QT].max())
            a[t] = min(l // 128, (S - win) // 128)
            if h > a[t] * 128 + win:
                ok = False
                break
        if ok:
            return pi, rank, kr, a, win
    raise ValueError("routes structure incompatible with banded-window kernel")


def _chunk_meta(a, win):
    """chunk -> (first tile, last tile); E-store block layout col=NCH*c + t."""
    NCH = win // 128
    a = [int(v) for v in a]
    chunk_tiles = {}
    for t in range(NT):
        for j in range(NCH):
            c = a[t] + j
            lo_t, hi_t = chunk_tiles.get(c, (t, t))
            chunk_tiles[c] = (min(lo_t, t), max(hi_t, t))
    blocks = {(c, t) for t in range(NT) for j in range(NCH) for c in (a[t] + j,)}
    idx = {NCH * c + t for (c, t) in blocks}
    if len(idx) != len(blocks):
        raise ValueError("E-store linear layout collision for this geometry")
    e_blocks = max(idx) + 1
    return chunk_tiles, e_blocks


def _build_module(a, win, loop_n=1, phases="ACD", cheat_dma=False):
    from contextlib import nullcontext

    from concourse import bacc, tile, mybir
    from concourse.bass import AP
    from concourse.masks import make_identity

    f32 = mybir.dt.float32
    bf16 = mybir.dt.bfloat16
    AF = mybir.ActivationFunctionType
    ALU = mybir.AluOpType
    NCH = win // 128                      # chunks per window
    KAP = win                             # E-store chunk stride (cols)
    a = [int(v) for v in a]
    chunk_tiles, e_blocks = _chunk_meta(a, win)
    E_COLS = e_blocks * 128

    nc = bacc.Bacc("TRN2", target_bir_lowering=False, debug=False)
    xT = nc.dram_tensor("xT", [DIM, S], bf16, kind="ExternalInput").ap()
    wqk = nc.dram_tensor("wqk", [DIM, 512], bf16, kind="ExternalInput").ap()
    bqk = nc.dram_tensor("bqk", [512, 1], f32, kind="ExternalInput").ap()
    wv = nc.dram_tensor("wv", [DIM, 256], bf16, kind="ExternalInput").ap()
    wo = nc.dram_tensor("wo", [256, DIM], bf16, kind="ExternalInput").ap()
    maskT = nc.dram_tensor("maskT", [128, NT * win], bf16, kind="ExternalInput").ap()
    outp = nc.dram_tensor("outp", [DIM, S], bf16, kind="ExternalOutput").ap()

    with tile.TileContext(nc) as tc:
        with tc.tile_pool(name="persist", bufs=1) as pp:
            id32 = pp.tile([128, 128], f32)
            make_identity(nc, id32)
            id_b = pp.tile([128, 128], bf16)
            nc.vector.tensor_copy(id_b, id32)
            ones1 = pp.tile([1, 128], bf16)
            nc.vector.memset(ones1, 1.0)

            # weights first (phase A gating), then mask (phase C), wo (phase D)
            wqk_sb = []
            for kk in range(8):
                wt = pp.tile([128, 512], bf16, tag=f"wqk{kk}", name=f"wqk{kk}")
                nc.sync.dma_start(out=wt, in_=wqk[kk * 128:(kk + 1) * 128, :])
                wqk_sb.append(wt)
            bq_sb = []
            for m in range(4):
                bt = pp.tile([128, 1], f32, tag=f"bqk{m}", name=f"bqk{m}")
                nc.sync.dma_start(out=bt, in_=bqk[m * 128:(m + 1) * 128, :])
                bq_sb.append(bt)
            wv_sb = []
            for kk in range(8):
                wt = pp.tile([128, 256], bf16, tag=f"wv{kk}", name=f"wv{kk}")
                nc.scalar.dma_start(out=wt, in_=wv[kk * 128:(kk + 1) * 128, :])
                wv_sb.append(wt)
            maskT_sb = pp.tile([128, NT * win], bf16)
            nc.scalar.dma_start(out=maskT_sb, in_=maskT)
            wo_sb = []
            for p2 in range(2):
                wt = pp.tile([128, DIM], bf16, tag=f"wo{p2}", name=f"wo{p2}")
                nc.scalar.dma_start(out=wt, in_=wo[p2 * 128:(p2 + 1) * 128, :])
                wo_sb.append(wt)

            qk_sb = [pp.tile([128, S], bf16, tag=f"qk{m}", name=f"qk{m}")
                     for m in range(4)]
            V65 = [pp.tile([128, HPC, 65], bf16, tag=f"V{cc}", name=f"V{cc}")
                   for cc in range(NT)]
            E_st = [pp.tile([128, E_COLS], bf16, tag=f"E{h}", name=f"E{h}")
                    for h in range(HPC)]
            attn_outT = [pp.tile([128, S], bf16, tag=f"aout{p}", name=f"aout{p}")
                         for p in range(2)]

            loop_cm = tc.For_i(0, loop_n, 1) if loop_n > 1 else nullcontext()
            with loop_cm:
                with tc.tile_pool(name="xt_pool", bufs=2) as pax, \
                     tc.tile_pool(name="st_pool", bufs=3) as pst, \
                     tc.tile_pool(name="rec_pool", bufs=2) as prc, \
                     tc.tile_pool(name="prb_pool", bufs=2) as prb_p, \
                     tc.tile_pool(name="psAV", bufs=2, space="PSUM") as psav, \
                     tc.tile_pool(name="psS", bufs=3, space="PSUM") as pss, \
                     tc.tile_pool(name="psPO", bufs=3, space="PSUM") as pspo:

                    xt_hold = [None]

                    def emit_A(n):
                        if "A" not in phases:
                            return
                        if cheat_dma and xt_hold[0] is not None:
                            xt = xt_hold[0]
                        else:
                            xt = []
                            for kk in range(8):
                                t_ = pax.tile([128, 512], bf16, tag=f"x{kk}",
                                              name=f"x{kk}_{n}")
                                nc.sync.dma_start(
                                    out=t_,
                                    in_=xT[kk * 128:(kk + 1) * 128,
                                           n * 512:(n + 1) * 512])
                                xt.append(t_)
                            xt_hold[0] = xt
                        for m in range(4):
                            ps = psav.tile([128, 512], f32, tag="av",
                                           name=f"psqk{m}_{n}")
                            for kk in range(8):
                                nc.tensor.matmul(
                                    ps, wqk_sb[kk][:, m * 128:(m + 1) * 128],
                                    xt[kk], start=(kk == 0), stop=(kk == 7))
                            nc.scalar.activation(
                                out=qk_sb[m][:, n * 512:(n + 1) * 512],
                                in_=ps, func=AF.Identity, bias=bq_sb[m])
                        for ss in range(4):
                            cc = n * 4 + ss
                            ps = psav.tile([128, 512], f32, tag="av",
                                           name=f"psv{cc}")
                            psv = ps[:, 0:256]
                            for kk in range(8):
                                nc.tensor.matmul(
                                    psv, xt[kk][:, ss * 128:(ss + 1) * 128],
                                    wv_sb[kk], start=(kk == 0), stop=(kk == 7),
                                    skip_group_check=True)
                            nc.vector.tensor_copy(V65[cc][:, :, 0:64],
                                                  psv.rearrange("p (h d) -> p h d", h=4))
                            nc.gpsimd.memset(V65[cc][:, :, 64:65], 1.0)

                    def emit_s1(h, u):
                        qTh = qk_sb[h // 2]
                        kTh = qk_sb[2 + h // 2]
                        poff = (h % 2) * 64
                        for t in range(u * SUP, (u + 1) * SUP):
                            ps = pss.tile([128, 512], f32, tag="sc",
                                          name=f"sc{h}_{t}")
                            ps_s = ps[:, 0:win]
                            for j in range(NCH):
                                nc.tensor.matmul(
                                    ps_s[:, j * 128:(j + 1) * 128],
                                    kTh[poff:poff + 64,
                                        (a[t] + j) * 128:(a[t] + j + 1) * 128],
                                    qTh[poff:poff + 64, t * 128:(t + 1) * 128],
                                    start=(j == 0), stop=(j == NCH - 1),
                                    skip_group_check=True)
                            # exp -> E store, one strided op: block col = NCH*c + t
                            col0 = (NCH * a[t] + t) * 128
                            base = E_st[h][:, col0:col0 + 128]
                            out_ap = AP(base.tensor, base.offset,
                                        [list(base.ap[0]), [KAP, NCH], [1, 128]])
                            nc.scalar.activation(out=out_ap, in_=ps_s, func=AF.Exp)
                            # multiplicative {1,0} mask, in place on the E store
                            m01 = maskT_sb[:, t * win:(t + 1) * win].rearrange(
                                "p (j q) -> p j q", j=NCH)
                            eng = nc.vector if (h + t) % 2 == 0 else nc.gpsimd
                            eng.tensor_tensor(out_ap, out_ap, m01, ALU.mult)

                    def emit_s2(h, u):
                        # PV pieces: widest chunk start=True, straddlers split
                        tiles_u = range(u * SUP, (u + 1) * SUP)
                        chunks_u = sorted({a[t] + j for t in tiles_u
                                           for j in range(NCH)})
                        ranges = []
                        for c in chunks_u:
                            t0c, t1c = chunk_tiles[c]
                            tlo = max(t0c, u * SUP)
                            thi = min(t1c, (u + 1) * SUP - 1)
                            ranges.append((c, tlo * 128 - u * 512,
                                           (thi + 1) * 128 - u * 512))
                        first = max(ranges, key=lambda r: r[2] - r[1])
                        pieces = [first]
                        wlo, whi = first[1], first[2]
                        for c, o0, o1 in sorted(
                                (r for r in ranges if r is not first),
                                key=lambda r: r[1]):
                            for p0, p1 in ((o0, min(o1, wlo)),
                                           (max(o0, wlo), min(o1, whi)),
                                           (max(o0, whi), o1)):
                                if p1 > p0:
                                    pieces.append((c, p0, p1))
                            wlo, whi = min(wlo, o0), max(whi, o1)
                        po = pspo.tile([65, 512], f32, tag="po",
                                       name=f"po{h}_{u}")
                        for i_p, (c, o0, o1) in enumerate(pieces):
                            e0 = KAP * c + o0 + u * 512
                            nc.tensor.matmul(
                                po[:, o0:o1],
                                V65[c][:, h, :],
                                E_st[h][:, e0:e0 + (o1 - o0)],
                                start=(i_p == 0),
                                stop=(i_p == len(pieces) - 1),
                                skip_group_check=True)
                        rec = prc.tile([1, 512], f32, tag="rec",
                                       name=f"rec{h}_{u}")
                        nc.vector.reciprocal(rec, po[64:65, :])
                        prb = prb_p.tile([64, 512], f32, tag="prb",
                                         name=f"prb{h}_{u}")
                        nc.gpsimd.partition_broadcast(prb, rec)
                        poff = (h % 2) * 64
                        nc.vector.tensor_tensor(
                            attn_outT[h // 2][poff:poff + 64,
                                              u * 512:(u + 1) * 512],
                            po[0:64, :], prb, ALU.mult)

                    def emit_D(nb):
                        if "D" not in phases:
                            return
                        for mm in range(8):
                            ps = psav.tile([128, 512], f32, tag="av",
                                           name=f"psd{mm}_{nb}")
                            for p2 in range(2):
                                nc.tensor.matmul(
                                    ps, wo_sb[p2][:, mm * 128:(mm + 1) * 128],
                                    attn_outT[p2][:, nb * 512:(nb + 1) * 512],
                                    start=(p2 == 0), stop=(p2 == 1))
                            st = pst.tile([128, 512], bf16, tag="st",
                                          name=f"st{mm}_{nb}")
                            if (mm + nb) % 2 == 0:
                                nc.scalar.copy(st, ps)
                            else:
                                nc.vector.tensor_copy(st, ps)
                            nc.sync.dma_start(
                                out=outp[mm * 128:(mm + 1) * 128,
                                         nb * 512:(nb + 1) * 512],
                                in_=st)

                    emit_A(0)
                    emit_A(1)
                    if "C" in phases:
                        for h in range(HPC):
                            emit_s1(h, 0)
                        emit_A(2)
                        for h in range(HPC):
                            emit_s2(h, 0)
                            emit_s1(h, 1)
                        emit_A(3)
                        for h in range(HPC):
                            emit_s2(h, 1)
                            emit_s1(h, 2)
                        emit_D(0)
                        for h in range(HPC):
                            emit_s2(h, 2)
                            emit_s1(h, 3)
                        emit_D(1)
                        emit_D(2)
                        for h in range(HPC):
                            emit_s2(h, 3)
                        emit_D(3)
                    else:
                        emit_A(2)
                        emit_A(3)

    nc.compile()
    return nc


def _get_module(a, win):
    key = (tuple(int(v) for v in a), int(win))
    if key not in _CACHE:
        _CACHE[key] = _build_module(a, win)
    return _CACHE[key]


def _prepare_in_maps(x, routes, qkv_w, qkv_b, out_w, out_b):
    """Shared host-side prep: returns (in_maps, pi, a, win)."""
    import ml_dtypes

    bf = ml_dtypes.bfloat16
    x = np.ascontiguousarray(np.asarray(x, np.float32))
    qkv_w = np.asarray(qkv_w, np.float32)
    qkv_b = np.asarray(qkv_b, np.float32)
    out_w = np.asarray(out_w, np.float32)

    pi, rank, kr, a, win = _geometry(np.asarray(routes))
    NCH = win // 128
    SCALE = 1.0 / float(np.sqrt(DH))

    # maskT [128, NT*win]: maskT[p, t*win + j*128 + q] = multiplicative mask
    # for key (a[t]+j)*128+p, query t*128+q (1 selected / 0 not)
    mask3 = np.zeros((NT, QT, win), np.float32)
    rows = np.repeat(np.arange(QT), K_NEI)
    for t in range(NT):
        krt = (kr[t * QT:(t + 1) * QT] - a[t] * 128).ravel()
        mask3[t, rows, krt] = 1.0
    maskT_np = np.ascontiguousarray(
        mask3.reshape(NT, QT, NCH, 128).transpose(3, 0, 2, 1).reshape(128, NT * win)
    ).astype(bf)

    xT_b = [np.ascontiguousarray(x[b][pi].T).astype(bf) for b in range(B)]

    in_maps = []
    for c in range(N_CORES):
        b = c // (N_CORES // B)
        hb = c % (N_CORES // B)
        heads = range(hb * HPC, (hb + 1) * HPC)
        w_rows, b_rows = [], []
        for sect, scale in ((0, SCALE), (1, 1.0)):
            for h in heads:
                r0 = sect * DIM + h * DH
                w_rows.append(qkv_w[r0:r0 + DH] * scale)
                b_rows.append(qkv_b[r0:r0 + DH] * scale)
        wqk_c = np.ascontiguousarray(np.concatenate(w_rows, 0).T).astype(bf)
        bqk_c = np.concatenate(b_rows, 0).reshape(-1, 1).astype(np.float32)
        vr0 = 2 * DIM + hb * HPC * DH
        wv_c = np.ascontiguousarray(qkv_w[vr0:vr0 + 256].T).astype(bf)
        wo_c = np.ascontiguousarray(
            out_w[:, hb * HPC * DH:(hb + 1) * HPC * DH].T).astype(bf)
        in_maps.append({
            "xT": xT_b[b],
            "wqk": wqk_c,
            "bqk": bqk_c,
            "wv": wv_c,
            "wo": wo_c,
            "maskT": maskT_np,
        })
    return in_maps, pi, a, win


def kernel(x, routes, qkv_w, qkv_b, out_w, out_b):
    from concourse.bass_utils import run_bass_kernel_spmd

    out_b = np.asarray(out_b, np.float32)
    qkv_b = np.asarray(qkv_b, np.float32)
    out_w = np.asarray(out_w, np.float32)
    in_maps, pi, a, win = _prepare_in_maps(x, routes, qkv_w, qkv_b, out_w, out_b)

    nc = _get_module(a, win)
    res = run_bass_kernel_spmd(nc, in_maps, core_ids=list(range(N_CORES)))

    # v-bias contribution: probs sum to 1, so attn@(v+bv) = attn@v + bv and
    # outp picks up a constant wo.T @ bv per core -- add on host.
    bv_all = qkv_b[2 * DIM:3 * DIM]
    adj = np.zeros(DIM, np.float64)
    for hb in range(N_CORES // B):
        sl = slice(hb * HPC * DH, (hb + 1) * HPC * DH)
        wo_c = np.asarray(in_maps[hb]["wo"], np.float64)     # [256, DIM] (bf16-rounded)
        adj += bv_all[sl].astype(np.float64) @ wo_c

    out = np.empty((B, S, DIM), np.float32)
    for b in range(B):
        cores = [c for c in range(N_CORES) if c // (N_CORES // B) == b]
        outT = res.results[cores[0]]["outp"].astype(np.float32)
        for c in cores[1:]:
            outT = outT + res.results[c]["outp"].astype(np.float32)
        rows_sorted = outT.T                      # [S, DIM] in rank order
        tmp = np.empty_like(rows_sorted)
        tmp[pi] = rows_sorted
        out[b] = tmp + (out_b.astype(np.float64) + adj)[None, :].astype(np.float32)
    return out


# revision 6
# speedup vs baseline: 1.5282x; 1.0798x over previous
"""Trainium2 Bass kernel for CantorAttention (transposed-softmax scheme).

Strategy
--------
Sorting positions by Cantor value makes every query's 64-key route set live
inside a 384-wide, 128-aligned window of the sorted order (dense banded
attention after a host-side permutation).  All matmuls run in bf16 (fp32
psum accumulation); rel err ~4e-3 vs the 2e-2 gate.

Per core (batch b, 4-head block hb), everything feature-major ("T" = [feat, seq]):

  A:  qkT = wqk.T @ xT (+bias, ACT copy to bf16); V produced directly in
      [seq, head, dim] layout (stationary = xT chunk, moving = wv), with a
      per-head ones column appended -> V65 [128, 4, 65].
  C:  scoresT per (head, 128-query tile): mask matmul FIRST (start=True,
      whole [128,384] psum = one zero-region), then 3 K^T.Q chunk matmuls
      accumulate.  ACT exp writes bf16 straight into a chunk-major E store
      (col = 384*chunk + 128*tile, collision-free for this geometry) via a
      strided AP -- no PE transposes, no DVE prob copies.
  PV: lhsT = V65[c][:,h,:] (65 rows: 64 v-dims + ones), rhs = E slices ->
      po[65, 512] = [unnormalized attn outT; denominators].  Normalize after:
      ACT copy po->SBUF, DVE reciprocal of the den row, Pool (gpsimd)
      partition_broadcast, DVE multiply -> attn_outT bf16.
  D:  outp = wo.T @ attn_outT per 512-col block, interleaved with C so the
      tail is short.

Sharding: batch x head-block -> 8 cores (core c: b = c//4, heads 4*(c%4)..).
Host sums the 4 partial outT blocks per batch, transposes, un-permutes, +bias.
"""

import sys

sys.path.insert(0, "/opt/trn_rl_repo")

import numpy as np

B, S, DIM = 2, 2048, 1024
HEADS, DH = 16, 64
K_NEI = 64
N_CORES = 8
HPC = 4            # heads per core
QT = 128           # query tile (rows per tile)
NT = S // QT       # 16 query tiles
SUP = 4            # query tiles per supertile (512 queries)
NSUP = NT // SUP

_CACHE = {}


def _cantor_val(seq_len, depth=8):
    pos = np.arange(seq_len, dtype=np.float64)
    x = pos / max(1, seq_len - 1)
    x = np.clip(x, 1e-6, 1.0 - 1e-6)
    val = np.zeros_like(x)
    factor = 0.5
    for _ in range(depth):
        xs = x * 3.0
        digit = np.floor(xs)
        x = xs - digit
        val = val + (digit == 2.0).astype(np.float64) * factor
        factor *= 0.5
    return np.clip(val, 0.0, 1.0)


def _geometry(routes):
    """Window geometry from the runtime routes array."""
    val = _cantor_val(S)
    pi = np.argsort(val, kind="stable").astype(np.int64)
    rank = np.empty(S, np.int64)
    rank[pi] = np.arange(S)
    kr = rank[np.asarray(routes, np.int64)][pi]      # [S, K] key ranks, query-rank order
    lo = kr.min(1)
    hi = kr.max(1) + 1
    for win in (384, 512):
        a = np.zeros(NT, np.int64)
        ok = True
        for t in range(NT):
            l = int(lo[t * QT:(t + 1) * QT].min())
            h = int(hi[t * QT:(t + 1) * QT].max())
            a[t] = min(l // 128, (S - win) // 128)
            if h > a[t] * 128 + win:
                ok = False
                break
        if ok:
            return pi, rank, kr, a, win
    raise ValueError("routes structure incompatible with banded-window kernel")


def _chunk_meta(a, win):
    """chunk -> (first tile, last tile); E-store block layout col=NCH*c + t."""
    NCH = win // 128
    a = [int(v) for v in a]
    chunk_tiles = {}
    for t in range(NT):
        for j in range(NCH):
            c = a[t] + j
            lo_t, hi_t = chunk_tiles.get(c, (t, t))
            chunk_tiles[c] = (min(lo_t, t), max(hi_t, t))
    blocks = {(c, t) for t in range(NT) for j in range(NCH) for c in (a[t] + j,)}
    idx = {NCH * c + t for (c, t) in blocks}
    if len(idx) != len(blocks):
        raise ValueError("E-store linear layout collision for this geometry")
    e_blocks = max(idx) + 1
    return chunk_tiles, e_blocks


def _build_module(a, win, loop_n=1, phases="ACD", cheat_dma=False):
    from contextlib import nullcontext

    from concourse import bacc, tile, mybir
    from concourse.bass import AP

    f32 = mybir.dt.float32
    bf16 = mybir.dt.bfloat16
    AF = mybir.ActivationFunctionType
    ALU = mybir.AluOpType
    NCH = win // 128                      # chunks per window
    KAP = win                             # E-store chunk stride (cols)
    a = [int(v) for v in a]
    chunk_tiles, e_blocks = _chunk_meta(a, win)
    E_COLS = e_blocks * 128

    nc = bacc.Bacc("TRN2", target_bir_lowering=False, debug=False)
    xT = nc.dram_tensor("xT", [DIM, S], bf16, kind="ExternalInput").ap()
    wqk = nc.dram_tensor("wqk", [DIM, 512], bf16, kind="ExternalInput").ap()
    bqk = nc.dram_tensor("bqk", [512, 1], f32, kind="ExternalInput").ap()
    wv = nc.dram_tensor("wv", [DIM, 256], bf16, kind="ExternalInput").ap()
    wo = nc.dram_tensor("wo", [256, DIM], bf16, kind="ExternalInput").ap()
    maskT = nc.dram_tensor("maskT", [128, NT * win], bf16, kind="ExternalInput").ap()
    outp = nc.dram_tensor("outp", [DIM, S], bf16, kind="ExternalOutput").ap()

    with tile.TileContext(nc) as tc:
        with tc.tile_pool(name="persist", bufs=1) as pp:
            # Batched DMAs (the issuing sequencer is held for the whole
            # transfer + ~900ns sem overhead, so fewer/bigger is critical).
            # SP queue: wqk, wv, maskT, wo (+ xt n>=1 and even-nb stores,
            # emitted inside the loop).  ACT queue: xt0, bqk, odd-nb stores.
            wqk_sb = pp.tile([128, 8, 512], bf16)
            nc.sync.dma_start(out=wqk_sb,
                              in_=wqk.rearrange("(kk p) n -> p kk n", p=128))
            wv_sb = pp.tile([128, 8, 256], bf16)
            nc.sync.dma_start(out=wv_sb,
                              in_=wv.rearrange("(kk p) n -> p kk n", p=128))
            maskT_sb = pp.tile([128, NT * win], bf16)
            nc.sync.dma_start(out=maskT_sb, in_=maskT)
            wo_sb2 = pp.tile([128, 2, DIM], bf16)
            nc.sync.dma_start(out=wo_sb2,
                              in_=wo.rearrange("(p2 p) n -> p p2 n", p=128))
            bq_sb = pp.tile([128, 4], f32)
            nc.scalar.dma_start(out=bq_sb,
                                in_=bqk.rearrange("(m p) o -> p (m o)", p=128))

            qk_sb = [pp.tile([128, S], bf16, tag=f"qk{m}", name=f"qk{m}")
                     for m in range(4)]
            V65 = [pp.tile([128, HPC, 65], bf16, tag=f"V{cc}", name=f"V{cc}")
                   for cc in range(NT)]
            E_st = [pp.tile([128, E_COLS], bf16, tag=f"E{h}", name=f"E{h}")
                    for h in range(HPC)]
            attn_outT = [pp.tile([128, S], bf16, tag=f"aout{p}", name=f"aout{p}")
                         for p in range(2)]

            loop_cm = tc.For_i(0, loop_n, 1) if loop_n > 1 else nullcontext()
            with loop_cm:
                with tc.tile_pool(name="xt_pool", bufs=2) as pax, \
                     tc.tile_pool(name="st_pool", bufs=3) as pst, \
                     tc.tile_pool(name="rec_pool", bufs=2) as prc, \
                     tc.tile_pool(name="prb_pool", bufs=2) as prb_p, \
                     tc.tile_pool(name="psAV", bufs=2, space="PSUM") as psav, \
                     tc.tile_pool(name="psS", bufs=3, space="PSUM") as pss, \
                     tc.tile_pool(name="psPO", bufs=3, space="PSUM") as pspo:

                    xt_hold = [None]

                    def emit_A(n):
                        if "A" not in phases:
                            return
                        if cheat_dma and xt_hold[0] is not None:
                            xt = xt_hold[0]
                        else:
                            xt = pax.tile([128, 8, 512], bf16, tag="x",
                                          name=f"x_{n}")
                            q_eng = nc.scalar if n == 0 else nc.sync
                            q_eng.dma_start(
                                out=xt,
                                in_=xT.rearrange("(kk p) n -> p kk n", p=128)
                                      [:, :, n * 512:(n + 1) * 512])
                            xt_hold[0] = xt
                        for m in range(4):
                            ps = psav.tile([128, 512], f32, tag="av",
                                           name=f"psqk{m}_{n}")
                            for kk in range(8):
                                nc.tensor.matmul(
                                    ps, wqk_sb[:, kk, m * 128:(m + 1) * 128],
                                    xt[:, kk, :], start=(kk == 0), stop=(kk == 7))
                            nc.scalar.activation(
                                out=qk_sb[m][:, n * 512:(n + 1) * 512],
                                in_=ps, func=AF.Identity, bias=bq_sb[:, m:m + 1])
                        for ss in range(4):
                            cc = n * 4 + ss
                            ps = psav.tile([128, 512], f32, tag="av",
                                           name=f"psv{cc}")
                            psv = ps[:, 0:256]
                            for kk in range(8):
                                nc.tensor.matmul(
                                    psv, xt[:, kk, ss * 128:(ss + 1) * 128],
                                    wv_sb[:, kk, :], start=(kk == 0), stop=(kk == 7),
                                    skip_group_check=True)
                            nc.vector.tensor_copy(V65[cc][:, :, 0:64],
                                                  psv.rearrange("p (h d) -> p h d", h=4))
                            nc.gpsimd.memset(V65[cc][:, :, 64:65], 1.0)

                    def emit_s1(h, u):
                        qTh = qk_sb[h // 2]
                        kTh = qk_sb[2 + h // 2]
                        poff = (h % 2) * 64
                        for t in range(u * SUP, (u + 1) * SUP):
                            ps = pss.tile([128, 512], f32, tag="sc",
                                          name=f"sc{h}_{t}")
                            ps_s = ps[:, 0:win]
                            for j in range(NCH):
                                nc.tensor.matmul(
                                    ps_s[:, j * 128:(j + 1) * 128],
                                    kTh[poff:poff + 64,
                                        (a[t] + j) * 128:(a[t] + j + 1) * 128],
                                    qTh[poff:poff + 64, t * 128:(t + 1) * 128],
                                    start=(j == 0), stop=(j == NCH - 1),
                                    skip_group_check=True)
                            # exp -> E store, one strided op: block col = NCH*c + t
                            col0 = (NCH * a[t] + t) * 128
                            base = E_st[h][:, col0:col0 + 128]
                            out_ap = AP(base.tensor, base.offset,
                                        [list(base.ap[0]), [KAP, NCH], [1, 128]])
                            nc.scalar.activation(out=out_ap, in_=ps_s, func=AF.Exp)
                            # multiplicative {1,0} mask, in place on the E store
                            m01 = maskT_sb[:, t * win:(t + 1) * win].rearrange(
                                "p (j q) -> p j q", j=NCH)
                            eng = nc.vector if (h + t) % 2 == 0 else nc.gpsimd
                            eng.tensor_tensor(out_ap, out_ap, m01, ALU.mult)

                    def emit_s2(h, u):
                        # PV pieces: widest chunk start=True, straddlers split
                        tiles_u = range(u * SUP, (u + 1) * SUP)
                        chunks_u = sorted({a[t] + j for t in tiles_u
                                           for j in range(NCH)})
                        ranges = []
                        for c in chunks_u:
                            t0c, t1c = chunk_tiles[c]
                            tlo = max(t0c, u * SUP)
                            thi = min(t1c, (u + 1) * SUP - 1)
                            ranges.append((c, tlo * 128 - u * 512,
                                           (thi + 1) * 128 - u * 512))
                        first = max(ranges, key=lambda r: r[2] - r[1])
                        pieces = [first]
                        wlo, whi = first[1], first[2]
                        for c, o0, o1 in sorted(
                                (r for r in ranges if r is not first),
                                key=lambda r: r[1]):
                            for p0, p1 in ((o0, min(o1, wlo)),
                                           (max(o0, wlo), min(o1, whi)),
                                           (max(o0, whi), o1)):
                                if p1 > p0:
                                    pieces.append((c, p0, p1))
                            wlo, whi = min(wlo, o0), max(whi, o1)
                        po = pspo.tile([65, 512], f32, tag="po",
                                       name=f"po{h}_{u}")
                        for i_p, (c, o0, o1) in enumerate(pieces):
                            e0 = KAP * c + o0 + u * 512
                            nc.tensor.matmul(
                                po[:, o0:o1],
                                V65[c][:, h, :],
                                E_st[h][:, e0:e0 + (o1 - o0)],
                                start=(i_p == 0),
                                stop=(i_p == len(pieces) - 1),
                                skip_group_check=True)
                        rec = prc.tile([1, 512], f32, tag="rec",
                                       name=f"rec{h}_{u}")
                        nc.vector.reciprocal(rec, po[64:65, :])
                        prb = prb_p.tile([64, 512], f32, tag="prb",
                                         name=f"prb{h}_{u}")
                        nc.gpsimd.partition_broadcast(prb, rec)
                        poff = (h % 2) * 64
                        nc.vector.tensor_tensor(
                            attn_outT[h // 2][poff:poff + 64,
                                              u * 512:(u + 1) * 512],
                            po[0:64, :], prb, ALU.mult)

                    outp3 = outp.rearrange("(mm p) n -> p mm n", p=128)

                    def emit_D(nb):
                        if "D" not in phases:
                            return
                        for half in range(2):
                            st = pst.tile([128, 4, 512], bf16, tag="st",
                                          name=f"st{half}_{nb}")
                            for i in range(4):
                                mm = half * 4 + i
                                ps = psav.tile([128, 512], f32, tag="av",
                                               name=f"psd{mm}_{nb}")
                                for p2 in range(2):
                                    nc.tensor.matmul(
                                        ps, wo_sb2[:, p2, mm * 128:(mm + 1) * 128],
                                        attn_outT[p2][:, nb * 512:(nb + 1) * 512],
                                        start=(p2 == 0), stop=(p2 == 1))
                                if (mm + nb) % 2 == 0:
                                    nc.scalar.copy(st[:, i, :], ps)
                                else:
                                    nc.vector.tensor_copy(st[:, i, :], ps)
                            q_eng = nc.sync if nb % 2 == 0 else nc.scalar
                            q_eng.dma_start(
                                out=outp3[:, half * 4:(half + 1) * 4,
                                          nb * 512:(nb + 1) * 512],
                                in_=st)

                    emit_A(0)
                    emit_A(1)
                    if "C" in phases:
                        for h in range(HPC):
                            emit_s1(h, 0)
                        emit_A(2)
                        for h in range(HPC):
                            emit_s2(h, 0)
                            emit_s1(h, 1)
                        emit_A(3)
                        for h in range(HPC):
                            emit_s2(h, 1)
                            emit_s1(h, 2)
                        emit_D(0)
                        for h in range(HPC):
                            emit_s2(h, 2)
                            emit_s1(h, 3)
                        emit_D(1)
                        emit_D(2)
                        for h in range(HPC):
                            emit_s2(h, 3)
                        emit_D(3)
                    else:
                        emit_A(2)
                        emit_A(3)

    nc.compile()
    return nc


def _get_module(a, win):
    key = (tuple(int(v) for v in a), int(win))
    if key not in _CACHE:
        _CACHE[key] = _build_module(a, win)
    return _CACHE[key]


def _prepare_in_maps(x, routes, qkv_w, qkv_b, out_w, out_b):
    """Shared host-side prep: returns (in_maps, pi, a, win)."""
    import ml_dtypes

    bf = ml_dtypes.bfloat16
    x = np.ascontiguousarray(np.asarray(x, np.float32))
    qkv_w = np.asarray(qkv_w, np.float32)
    qkv_b = np.asarray(qkv_b, np.float32)
    out_w = np.asarray(out_w, np.float32)

    pi, rank, kr, a, win = _geometry(np.asarray(routes))
    NCH = win // 128
    SCALE = 1.0 / float(np.sqrt(DH))

    # maskT [128, NT*win]: maskT[p, t*win + j*128 + q] = multiplicative mask
    # for key (a[t]+j)*128+p, query t*128+q (1 selected / 0 not)
    mask3 = np.zeros((NT, QT, win), np.float32)
    rows = np.repeat(np.arange(QT), K_NEI)
    for t in range(NT):
        krt = (kr[t * QT:(t + 1) * QT] - a[t] * 128).ravel()
        mask3[t, rows, krt] = 1.0
    maskT_np = np.ascontiguousarray(
        mask3.reshape(NT, QT, NCH, 128).transpose(3, 0, 2, 1).reshape(128, NT * win)
    ).astype(bf)

    xT_b = [np.ascontiguousarray(x[b][pi].T).astype(bf) for b in range(B)]

    in_maps = []
    for c in range(N_CORES):
        b = c // (N_CORES // B)
        hb = c % (N_CORES // B)
        heads = range(hb * HPC, (hb + 1) * HPC)
        w_rows, b_rows = [], []
        for sect, scale in ((0, SCALE), (1, 1.0)):
            for h in heads:
                r0 = sect * DIM + h * DH
                w_rows.append(qkv_w[r0:r0 + DH] * scale)
                b_rows.append(qkv_b[r0:r0 + DH] * scale)
        wqk_c = np.ascontiguousarray(np.concatenate(w_rows, 0).T).astype(bf)
        bqk_c = np.concatenate(b_rows, 0).reshape(-1, 1).astype(np.float32)
        vr0 = 2 * DIM + hb * HPC * DH
        wv_c = np.ascontiguousarray(qkv_w[vr0:vr0 + 256].T).astype(bf)
        wo_c = np.ascontiguousarray(
            out_w[:, hb * HPC * DH:(hb + 1) * HPC * DH].T).astype(bf)
        in_maps.append({
            "xT": xT_b[b],
            "wqk": wqk_c,
            "bqk": bqk_c,
            "wv": wv_c,
            "wo": wo_c,
            "maskT": maskT_np,
        })
    return in_maps, pi, a, win


def kernel(x, routes, qkv_w, qkv_b, out_w, out_b):
    from concourse.bass_utils import run_bass_kernel_spmd

    out_b = np.asarray(out_b, np.float32)
    qkv_b = np.asarray(qkv_b, np.float32)
    out_w = np.asarray(out_w, np.float32)
    in_maps, pi, a, win = _prepare_in_maps(x, routes, qkv_w, qkv_b, out_w, out_b)

    nc = _get_module(a, win)
    res = run_bass_kernel_spmd(nc, in_maps, core_ids=list(range(N_CORES)))

    # v-bias contribution: probs sum to 1, so attn@(v+bv) = attn@v + bv and
    # outp picks up a constant wo.T @ bv per core -- add on host.
    bv_all = qkv_b[2 * DIM:3 * DIM]
    adj = np.zeros(DIM, np.float64)
    for hb in range(N_CORES // B):
        sl = slice(hb * HPC * DH, (hb + 1) * HPC * DH)
        wo_c = np.asarray(in_maps[hb]["wo"], np.float64)     # [256, DIM] (bf16-rounded)
        adj += bv_all[sl].astype(np.float64) @ wo_c

    out = np.empty((B, S, DIM), np.float32)
    for b in range(B):
        cores = [c for c in range(N_CORES) if c // (N_CORES // B) == b]
        outT = res.results[cores[0]]["outp"].astype(np.float32)
        for c in cores[1:]:
            outT = outT + res.results[c]["outp"].astype(np.float32)
        rows_sorted = outT.T                      # [S, DIM] in rank order
        tmp = np.empty_like(rows_sorted)
        tmp[pi] = rows_sorted
        out[b] = tmp + (out_b.astype(np.float64) + adj)[None, :].astype(np.float32)
    return out
